# revision 1
# baseline (speedup 1.0000x reference)
"""Trainium2 Bass kernel for nn_BiAttention (sparse_attention).

Math: the reference's attention matrix is rank-1 plus a mask bias:
    att[b,l,m] = input_dot[b,l] + s[b,m],  s[m] = memory[m]@w_mem1 - 1e30*(1-mask[m])
Row softmax over m is invariant to the per-row constant input_dot[b,l], so
    weight_one[b,l,:] = softmax_m(s)            (same for every l)
    output_one[b,l,:] = v_b := softmax_m(s) @ (memory @ W_mem2.T + b_mem2)
Likewise max_m att[b,l,m] = input_dot[b,l] + const, so
    weight_two[b,0,:] = softmax_l(input_dot)
    output_two[b,0,:] = softmax_l(input_dot) @ inp2
The output [N, 4*Ld, d] row blocks are:
    [0:2048]    inp2 = input @ W_in2.T + b_in2
    [2048:4096] v_b broadcast
    [4096:6144] inp2 * v_b
    [6144:8192] (output_two * v_b) broadcast

Sharding: pure data parallel, one batch element per NeuronCore (8 cores).

Schedule notes: engine sequencers and HWDGE rings are strict FIFO, so
emission order is scheduling. Reads go on the ACT DMA ring, writes on
the SP ring. The big matmul runs in float32r (full PE rate, ~1e-4 rel).
The v path (W_mem2) is computed via PE transposes + f32r matvecs and is
spliced between the first main-loop tiles; prod (inp2*v) is deferred a
few tiles so the DVE FIFO never blocks on v_bc.
"""

import numpy as np

import concourse.bass as bass
import concourse.tile as tile
from concourse import bacc, mybir
from concourse.bass_utils import run_bass_kernel_spmd
from concourse.masks import make_identity

F32 = mybir.dt.float32
F32R = mybir.dt.float32r
AX = mybir.AxisListType
OP = mybir.AluOpType
EXP = mybir.ActivationFunctionType.Exp

P = 128
BSZ, LD, LM, HID = 8, 2048, 512, 1024
KT = HID // P          # 8 hidden-dim chunks
LT = LD // P           # 16 l tiles
MT = LM // P           # 4 memory tiles
N_CORES = 8

MM_SKEW = 1            # matmuls for tile i emitted 1 iter after transposes
S_SKEW = 3             # S-accumulation matmuls trail by 3 iters
PROD_SKEW = 4          # prod (needs v_bc) trails by 7 iters

_NC_CACHE = None


def _build_nc():
    nc = bacc.Bacc("TRN2", target_bir_lowering=False, num_devices=N_CORES)

    inp_d = nc.dram_tensor("input", [LD, HID], F32, kind="ExternalInput").ap()
    mem_d = nc.dram_tensor("memory", [LM, HID], F32, kind="ExternalInput").ap()
    mask_d = nc.dram_tensor("mask", [1, LM], F32, kind="ExternalInput").ap()
    wi1_d = nc.dram_tensor("w_in1", [1, HID], F32, kind="ExternalInput").ap()
    wm1_d = nc.dram_tensor("w_mem1", [1, HID], F32, kind="ExternalInput").ap()
    Wi2_d = nc.dram_tensor("W_in2", [HID, HID], F32, kind="ExternalInput").ap()
    bi2_d = nc.dram_tensor("b_in2", [1, HID], F32, kind="ExternalInput").ap()
    Wm2_d = nc.dram_tensor("W_mem2", [HID, HID], F32, kind="ExternalInput").ap()
    bm2_d = nc.dram_tensor("b_mem2", [1, HID], F32, kind="ExternalInput").ap()
    out_d = nc.dram_tensor("out", [4 * LD, HID], F32, kind="ExternalOutput").ap()

    with tile.TileContext(nc) as tc:
        with (
            tc.tile_pool(name="const", bufs=1) as cpool,
            tc.tile_pool(name="bc", bufs=1) as bcpool,
            tc.tile_pool(name="wT", bufs=1) as wtpool,
            tc.tile_pool(name="wT8", bufs=8) as wt8pool,
            tc.tile_pool(name="mem", bufs=1) as mempool,
            tc.tile_pool(name="wstage", bufs=3) as wstagepool,
            tc.tile_pool(name="rows", bufs=1) as rowpool,
            tc.tile_pool(name="at", bufs=5) as atpool,
            tc.tile_pool(name="intp", bufs=3) as intpool,
            tc.tile_pool(name="inp2", bufs=7) as inp2pool,
            tc.tile_pool(name="prod", bufs=3) as prodpool,
            tc.tile_pool(name="ttr", bufs=2) as ttrpool,
            tc.tile_pool(name="small", bufs=4) as smallpool,
            tc.tile_pool(name="ptr", bufs=2, space="PSUM") as ptrpool,
            tc.tile_pool(name="pout", bufs=4, space="PSUM") as poutpool,
            tc.tile_pool(name="psS", bufs=1, space="PSUM") as pspool,
        ):
            # ---------------- constants & small loads (ACT ring) ----------
            ident = cpool.tile([P, P], F32)
            make_identity(nc, ident)
            ones_row = cpool.tile([1, P], F32)
            nc.vector.memset(ones_row[:], 1.0)
            ones_col = cpool.tile([P, 1], F32)
            nc.vector.memset(ones_col[:], 1.0)

            bm2_row = rowpool.tile([1, HID], F32, tag="bm2r")
            nc.scalar.dma_start(bm2_row[:], bm2_d[:])
            mask_col = cpool.tile([P, MT], F32)
            nc.scalar.dma_start(mask_col[:], mask_d.rearrange("1 (o p) -> p o", p=P))
            wm1_bc = bcpool.tile([P, HID], F32, tag="wm1bc")
            nc.scalar.dma_start(wm1_bc[:], wm1_d.to_broadcast([P, HID]))

            # ---------------- helpers ----------------
            at_tiles = {}
            int_ts = {}
            inp2_sbs = {}
            e_tile = cpool.tile([P, LT], F32R)
            e_f32 = cpool.tile([P, LT], F32)
            s_ps = [pspool.tile([1, 512], F32, tag=f"s{h}", name=f"s{h}")
                    for h in range(2)]

            def emit_at(i):
                at = atpool.tile([P, HID], F32, tag="at", name=f"at{i}")
                nc.scalar.dma_start(at[:], inp_d[i * P:(i + 1) * P, :])
                at_tiles[i] = at

            def emit_tr_idot(i):
                at = at_tiles.pop(i)
                int_t = intpool.tile([P, KT, P], F32R, tag="int", name=f"int{i}")
                for kh in range(2):
                    ps = ptrpool.tile([P, 512], F32, tag="tr", name=f"trp{i}_{kh}")
                    for j in range(4):
                        k = kh * 4 + j
                        nc.tensor.transpose(
                            ps[:, j * P:(j + 1) * P],
                            at[:, k * P:(k + 1) * P],
                            ident,
                        )
                    nc.scalar.copy(
                        int_t[:, kh * 4:(kh + 1) * 4, :].rearrange("p a b -> p (a b)"),
                        ps[:],
                    )
                int_ts[i] = int_t
                scr = ttrpool.tile([P, HID], F32, tag="ttr", name=f"scr{i}")
                idc = smallpool.tile([P, 1], F32, tag="idc", name=f"idc{i}")
                nc.vector.tensor_mul(scr[:], at[:], wi1_bc[:])
                nc.vector.tensor_reduce(idc[:], scr[:], AX.X, OP.add)
                nc.scalar.activation(e_f32[:, i:i + 1], idc[:], EXP)
                nc.scalar.copy(e_tile[:, i:i + 1], e_f32[:, i:i + 1])

            def emit_mm(i):
                int_t = int_ts.pop(i)
                inp2_sb = inp2pool.tile([P, HID], F32R, tag="inp2", name=f"i2_{i}")
                for h in range(2):
                    pso = poutpool.tile([P, 512], F32, tag="out", name=f"pso{i}_{h}")
                    for k in range(KT):
                        nc.tensor.matmul(
                            pso[:], int_t[:, k, :],
                            w2t[k][:, h * 512:(h + 1) * 512],
                            start=(k == 0), stop=(k == KT - 1),
                        )
                    nc.vector.tensor_add(
                        inp2_sb[:, h * 512:(h + 1) * 512], pso[:],
                        bi2_bc[:, h * 512:(h + 1) * 512],
                    )
                nc.sync.dma_start(
                    out_d[i * P:(i + 1) * P, :], inp2_sb.bitcast(F32)[:]
                )
                inp2_sbs[i] = inp2_sb

            def emit_s(i):
                for h in range(2):
                    nc.tensor.matmul(
                        s_ps[h][:], e_tile[:, i:i + 1],
                        inp2_sbs[i][:, h * 512:(h + 1) * 512],
                        start=(i == 0), stop=(i == LT - 1),
                        skip_group_check=True,
                    )

            def emit_prod(i):
                inp2_sb = inp2_sbs.pop(i)
                prod_sb = prodpool.tile([P, HID], F32, tag="prod", name=f"pr{i}")
                nc.vector.tensor_mul(prod_sb[:], inp2_sb.bitcast(F32)[:], v_bc[:])
                nc.sync.dma_start(
                    out_d[2 * LD + i * P:2 * LD + (i + 1) * P, :], prod_sb[:]
                )

            def transpose_1024(dst, src_nat, tag):
                """dst[:, k, oj*128: ] = transpose of src_nat[:, oj, k*128: ]."""
                for k in range(KT):
                    for ojh in range(2):
                        ps = ptrpool.tile([P, 512], F32, tag="tr",
                                          name=f"{tag}{k}_{ojh}")
                        for j in range(4):
                            oj = ojh * 4 + j
                            nc.tensor.transpose(
                                ps[:, j * P:(j + 1) * P],
                                src_nat[:, oj, k * P:(k + 1) * P],
                                ident,
                            )
                        nc.scalar.copy(
                            dst[:, k, ojh * 512:(ojh + 1) * 512], ps[:]
                        )

            def rank1_bcast(row_ap, name):
                bc = bcpool.tile([P, HID], F32, tag=name, name=name)
                for h in range(2):
                    ps = poutpool.tile([P, 512], F32, tag="out", name=f"{name}{h}")
                    nc.tensor.matmul(
                        ps[:], ones_row[:], row_ap[:, h * 512:(h + 1) * 512],
                        start=True, stop=True,
                    )
                    nc.scalar.copy(bc[:, h * 512:(h + 1) * 512], ps[:])
                return bc

            # ---------------- head: input tile 0, then W chunks ------------
            # W_in2 streams through small staging tiles; transposed per
            # k-chunk into 8 persistent w2t tiles (per-k dep granularity)
            wi2_r = Wi2_d.rearrange("(o p) d -> p o d", p=P)
            emit_at(0)
            w_stages = {}

            def wi2_stage(k):
                st = wstagepool.tile([P, KT, P], F32, tag="wst", name=f"wist{k}")
                nc.scalar.dma_start(st[:], wi2_r[:, :, k * P:(k + 1) * P])
                w_stages[k] = st

            for k in range(4):
                wi2_stage(k)
            emit_at(1)
            wi2_stage(4)
            wi2_stage(5)
            emit_at(2)
            wi2_stage(6)
            wi2_stage(7)
            wi1_bc = bcpool.tile([P, HID], F32, tag="wi1bc")
            nc.scalar.dma_start(wi1_bc[:], wi1_d.to_broadcast([P, HID]))
            bi2_bc = bcpool.tile([P, HID], F32, tag="bi2bc")
            nc.scalar.dma_start(bi2_bc[:], bi2_d.to_broadcast([P, HID]))

            # memory for the s path (on the write ring, which is idle early)
            mem_t = mempool.tile([P, MT, HID], F32, tag="memt")
            nc.sync.dma_start(mem_t[:], mem_d.rearrange("(j p) d -> p j d", p=P))

            # tile-0 transposes fill PE while W chunks stream in
            emit_tr_idot(0)
            # W_in2^T on PE, one k-chunk at a time
            w2t = []
            for k in range(KT):
                st = w_stages.pop(k)
                w2t_k = wt8pool.tile([P, HID], F32R, tag="w2t", name=f"w2t{k}")
                for ojh in range(2):
                    ps = ptrpool.tile([P, 512], F32, tag="tr", name=f"wt{k}_{ojh}")
                    for j in range(4):
                        oj = ojh * 4 + j
                        nc.tensor.transpose(
                            ps[:, j * P:(j + 1) * P],
                            st[:, oj, :],
                            ident,
                        )
                    nc.scalar.copy(w2t_k[:, ojh * 512:(ojh + 1) * 512], ps[:])
                w2t.append(w2t_k)
            emit_at(3)

            emit_tr_idot(1)
            emit_mm(0)

            # s path on DVE, smeared between early tiles to use DVE slack
            s_col = smallpool.tile([P, MT], F32, tag="scol")

            def s_mul(j):
                scr = ttrpool.tile([P, HID], F32, tag="ttr", name=f"sscr{j}")
                nc.vector.tensor_mul(scr[:], mem_t[:, j, :], wm1_bc[:])
                nc.vector.tensor_reduce(s_col[:, j:j + 1], scr[:], AX.X, OP.add)

            s_mul(0)
            s_mul(1)

            # W_mem2 staged per o-block on the read ring; transposed into
            # wm2t halves as blocks arrive (wstage slots recycle)
            wm2_r = Wm2_d.rearrange("(o p) d -> p o d", p=P)
            wm2_stages = {}
            for oj in range(4):
                st = wstagepool.tile([P, KT, P], F32, tag="wst", name=f"wst{oj}")
                nc.sync.dma_start(st[:], wm2_r[:, oj, :].rearrange("p (a b) -> p a b", b=P))
                wm2_stages[oj] = st
                if oj == 1:
                    emit_at(4)

            emit_tr_idot(2)
            emit_mm(1)
            s_mul(2)
            s_mul(3)
            msk = smallpool.tile([P, MT], F32, tag="msk")
            nc.vector.tensor_scalar(msk[:], mask_col[:], -1.0, 1e30, OP.add, OP.mult)
            nc.vector.tensor_add(s_col[:], s_col[:], msk[:])
            e_s = smallpool.tile([P, MT], F32, tag="es")
            nc.scalar.activation(e_s[:], s_col[:], EXP)

            emit_at(5)
            emit_tr_idot(3)
            emit_mm(2)
            emit_s(0)

            # ---------------- v path (consolidated PE block) --------------
            # P_un[o] = sum_m e_s[m] * memory[m, o]  (PE, fp32) and Z_s
            pun_row = rowpool.tile([1, HID], F32, tag="punr")
            for h in range(2):
                pun_ps = ptrpool.tile([1, 512], F32, tag="tr", name=f"pun{h}")
                for j in range(MT):
                    nc.tensor.matmul(
                        pun_ps[:], e_s[:, j:j + 1],
                        mem_t[:, j, h * 512:(h + 1) * 512],
                        start=(j == 0), stop=(j == MT - 1),
                        skip_group_check=True,
                    )
                nc.scalar.copy(pun_row[:, h * 512:(h + 1) * 512], pun_ps[:])
            zs_ps = poutpool.tile([1, MT], F32, tag="out")
            nc.tensor.matmul(zs_ps[:], ones_col[:], e_s[:], start=True, stop=True)
            zs_row = smallpool.tile([1, 1], F32, tag="zs")
            nc.vector.tensor_reduce(zs_row[:], zs_ps[:], AX.X, OP.add)
            rzs = smallpool.tile([1, 1], F32, tag="rzs")
            nc.vector.reciprocal(rzs[:], zs_row[:])

            # p as f32r column chunks [128, KT]
            p_col_f = smallpool.tile([P, KT], F32, tag="pcolf")
            for j in range(KT):
                nc.sync.dma_start(
                    p_col_f[:, j:j + 1], pun_row[:, j * P:(j + 1) * P]
                )
            p_col = smallpool.tile([P, KT], F32R, tag="pcol")
            nc.vector.tensor_copy(p_col[:], p_col_f[:])

            # v_unb[o] = sum_d p[d] * W_mem2^T[d, o]  (f32r matvecs), with
            # W_mem2^T built half-at-a-time to halve its SBUF footprint
            v_row = rowpool.tile([1, HID], F32, tag="vrow")
            for h in range(2):
                wm2t = wtpool.tile([P, KT, 512], F32R, tag="wm2t",
                                   name=f"wm2t{h}")
                for j in range(4):
                    oj = h * 4 + j
                    st = wm2_stages.pop(oj)
                    for kh in range(2):
                        ps = ptrpool.tile([P, 512], F32, tag="tr",
                                          name=f"wmt{oj}_{kh}")
                        for jj in range(4):
                            k = kh * 4 + jj
                            nc.tensor.transpose(
                                ps[:, jj * P:(jj + 1) * P],
                                st[:, k, :],
                                ident,
                            )
                        nc.scalar.copy(
                            wm2t[:, kh * 4:(kh + 1) * 4, j * P:(j + 1) * P],
                            ps.rearrange("p (a b) -> p a b", a=4),
                        )
                    # stage the second half's block while slots free up
                    if h == 0:
                        st2 = wstagepool.tile([P, KT, P], F32, tag="wst",
                                              name=f"wst{oj + 4}")
                        nc.sync.dma_start(
                            st2[:],
                            wm2_r[:, oj + 4, :].rearrange("p (a b) -> p a b", b=P),
                        )
                        wm2_stages[oj + 4] = st2
                v_ps = ptrpool.tile([1, 512], F32, tag="tr", name=f"vps{h}")
                for k in range(KT):
                    nc.tensor.matmul(
                        v_ps[:], p_col[:, k:k + 1],
                        wm2t[:, k, :],
                        start=(k == 0), stop=(k == KT - 1),
                        skip_group_check=True,
                    )
                nc.scalar.copy(v_row[:, h * 512:(h + 1) * 512], v_ps[:])
            nc.vector.tensor_scalar(v_row[:], v_row[:], rzs[:], None, OP.mult)
            nc.vector.tensor_add(v_row[:], v_row[:], bm2_row[:])
            v_bc = rank1_bcast(v_row, "vbc")

            # ---------------- steady-state loop ----------------
            # iteration i: at(i+2), tr/idot(i), mm(i-1), s(i-3), prod(i-5),
            # plus v-row writes spread out
            vw = [0]

            def emit_vwrite():
                i = vw[0]
                if i < LT:
                    nc.sync.dma_start(
                        out_d[LD + i * P:LD + (i + 1) * P, :], v_bc[:]
                    )
                    vw[0] += 1

            for i in range(4, LT + PROD_SKEW + 1):
                if i + 2 < LT:
                    emit_at(i + 2)
                if i < LT:
                    emit_tr_idot(i)
                if i - MM_SKEW < LT:
                    emit_mm(i - MM_SKEW)
                if 0 < i - S_SKEW < LT:
                    emit_s(i - S_SKEW)
                if 0 <= i - PROD_SKEW < LT:
                    emit_prod(i - PROD_SKEW)
                emit_vwrite()
                emit_vwrite()

            while vw[0] < LT:
                emit_vwrite()

            # ---------------- tail: out2 and u rows ----------------
            z_ps = poutpool.tile([1, LT], F32, tag="out")
            nc.tensor.matmul(z_ps[:], ones_col[:], e_f32[:], start=True, stop=True)
            z_row = smallpool.tile([1, LT], F32, tag="zrow")
            nc.scalar.copy(z_row[:], z_ps[:])
            z_sb = smallpool.tile([1, 1], F32, tag="z")
            nc.vector.tensor_reduce(z_sb[:], z_row[:], AX.X, OP.add)
            rz = smallpool.tile([1, 1], F32, tag="rz")
            nc.vector.reciprocal(rz[:], z_sb[:])
            s_row = rowpool.tile([1, HID], F32, tag="srow")
            for h in range(2):
                nc.scalar.copy(s_row[:, h * 512:(h + 1) * 512], s_ps[h][:])
            u_row = rowpool.tile([1, HID], F32, tag="urow")
            nc.vector.tensor_scalar(u_row[:], s_row[:], rz[:], None, OP.mult)
            nc.vector.tensor_mul(u_row[:], u_row[:], v_row[:])
            u_bc = rank1_bcast(u_row, "ubc")
            nc.sync.dma_start(
                out_d[3 * LD:4 * LD, :].rearrange("(t p) d -> p t d", p=P),
                u_bc[:, None, :].to_broadcast([P, LT, HID]),
            )

    nc.finalize()
    return nc


def _get_nc():
    global _NC_CACHE
    if _NC_CACHE is None:
        _NC_CACHE = _build_nc()
    return _NC_CACHE


def kernel(**inputs) -> np.ndarray:
    nc = _get_nc()
    inp = np.asarray(inputs["input"], dtype=np.float32)
    mem = np.asarray(inputs["memory"], dtype=np.float32)
    mask = np.asarray(inputs["mask"], dtype=np.float32)
    w_in1 = np.ascontiguousarray(np.asarray(inputs["w_in1"], np.float32).reshape(1, HID))
    w_mem1 = np.ascontiguousarray(np.asarray(inputs["w_mem1"], np.float32).reshape(1, HID))
    W_in2 = np.ascontiguousarray(np.asarray(inputs["W_in2"], np.float32))
    b_in2 = np.ascontiguousarray(np.asarray(inputs["b_in2"], np.float32).reshape(1, HID))
    W_mem2 = np.ascontiguousarray(np.asarray(inputs["W_mem2"], np.float32))
    b_mem2 = np.ascontiguousarray(np.asarray(inputs["b_mem2"], np.float32).reshape(1, HID))

    in_maps = []
    for b in range(N_CORES):
        in_maps.append({
            "input": np.ascontiguousarray(inp[b]),
            "memory": np.ascontiguousarray(mem[b]),
            "mask": np.ascontiguousarray(mask[b].reshape(1, LM)),
            "w_in1": w_in1,
            "w_mem1": w_mem1,
            "W_in2": W_in2,
            "b_in2": b_in2,
            "W_mem2": W_mem2,
            "b_mem2": b_mem2,
        })

    res = run_bass_kernel_spmd(nc, in_maps, core_ids=list(range(N_CORES)))
    return np.stack([res.results[c]["out"] for c in range(N_CORES)], axis=0)



# revision 24
# speedup vs baseline: 1.4653x; 1.4653x over previous
"""Trainium2 Bass kernel for nn_BiAttention (sparse_attention).

Math: the attention matrix is rank-1 plus a mask bias:
    att[b,l,m] = idot[b,l] + s_m[b,m]
Row softmax over m is invariant to the per-row constant idot[b,l], so
    output_one[b,l,:] = v_b := softmax_m(s_m) @ (memory @ W_mem2.T + b_mem2)
and max_m att = idot + const, so
    output_two[b,0,:] = softmax_l(idot) @ inp2 = (W_in2 @ q + Z*b_in2)/Z
    with q[d] = sum_l e_l * input[l,d],  Z = sum_l e_l,  e = exp(idot)
Output row blocks [N, 4*Ld, d]:
    [0:2048]    inp2 = input @ W_in2.T + b_in2        (full rank, device)
    [2048:4096] v_b broadcast                          (host replicates row)
    [4096:6144] inp2 * v_b                             (full rank, device)
    [6144:8192] (output_two * v_b) broadcast           (host replicates row)

Device computes everything TRANSPOSED (output features on partitions):
    i2T[o, l] = inp2[l, o] = sum_d W_in2^T[d,o] * input^T[d,l] + b[o]
so the bias is a per-partition scalar (ACT engine Identity+bias) and
prod = i2T * v[o] is a per-partition tensor_scalar on DVE. All matmul
operands are bf16 (inputs/weights pre-transposed + converted on host),
outputs stored bf16 and upconverted on host: rel err ~5e-3 << 2e-2.

Sharding: pure data parallel, one batch element per NeuronCore (8 cores).

Engine budget @2.4GHz PE: main matmul 131k cyc; everything else rides in
N=1 matmul columns (idot/p/v/s ~300 matmuls of ~1 cycle) or on DVE/ACT.
DMA: reads 9.5MB + writes 8.4MB ~= 50us < PE ~60us. Reads split across
ACT (weights) and Pool/SWDGE (input) rings; writes on SP ring.
"""

import numpy as np
import ml_dtypes

import concourse.bass as bass
import concourse.tile as tile
from concourse import bacc, mybir
from concourse.bass_utils import run_bass_kernel_spmd
from concourse.masks import make_identity

F32 = mybir.dt.float32
F32R = mybir.dt.float32r
BF16 = mybir.dt.bfloat16
AX = mybir.AxisListType
OP = mybir.AluOpType
EXP = mybir.ActivationFunctionType.Exp
IDENT = mybir.ActivationFunctionType.Identity
COPY = mybir.ActivationFunctionType.Copy

P = 128
BSZ, LD, LM, HID = 8, 2048, 512, 1024
KT = HID // P          # 8 hidden-dim chunks
LT = LD // P           # 16 l slices of 128
MT = LM // P           # 4 memory tiles
GT = 4                 # l groups of 512
GL = LD // GT          # 512
N_CORES = 8

_NC_CACHE = None

# bisection gate: 0=mm only, 1=+idot, 2=+tr/ebc, 3=+q, 4=+mem path,
# 5=+p/v, 6=+s/u, 7=full (prod)
import os as _os
_LEVEL = int(_os.environ.get("KLEVEL", "7"))


def _build_nc():
    nc = bacc.Bacc("TRN2", target_bir_lowering=False, num_devices=N_CORES)

    inT_d = nc.dram_tensor("inT", [HID, LD], BF16, kind="ExternalInput").ap()
    w2t_d = nc.dram_tensor("w2t", [HID, HID], BF16, kind="ExternalInput").ap()
    wm2t_d = nc.dram_tensor("wm2t", [HID, HID], BF16, kind="ExternalInput").ap()
    mem_d = nc.dram_tensor("memory", [LM, HID], BF16, kind="ExternalInput").ap()
    mask_d = nc.dram_tensor("mask", [1, LM], F32, kind="ExternalInput").ap()
    w1c_d = nc.dram_tensor("w1c", [P, KT], BF16, kind="ExternalInput").ap()
    wm1_d = nc.dram_tensor("wm1", [1, HID], BF16, kind="ExternalInput").ap()
    bi2c_d = nc.dram_tensor("bi2c", [P, KT], F32, kind="ExternalInput").ap()
    bm2c_d = nc.dram_tensor("bm2c", [P, KT], F32, kind="ExternalInput").ap()
    o1T_d = nc.dram_tensor("o1T", [HID, LD], BF16, kind="ExternalOutput").ap()
    o3T_d = nc.dram_tensor("o3T", [HID, LD], BF16, kind="ExternalOutput").ap()
    vrow_d = nc.dram_tensor("vrow", [P, KT], F32, kind="ExternalOutput").ap()
    urow_d = nc.dram_tensor("urow", [P, KT], F32, kind="ExternalOutput").ap()

    with tile.TileContext(nc) as tc:
        with (
            tc.tile_pool(name="const", bufs=1) as cpool,
            tc.tile_pool(name="w", bufs=1) as wpool,
            tc.tile_pool(name="inp", bufs=1) as inpool,
            tc.tile_pool(name="i2", bufs=1) as i2pool,
            tc.tile_pool(name="o3", bufs=3) as o3pool,
            tc.tile_pool(name="scr", bufs=2) as scrpool,
            tc.tile_pool(name="sm", bufs=1) as smpool,
            tc.tile_pool(name="psmm", bufs=3, space="PSUM") as psmm,
            tc.tile_pool(name="psbc", bufs=1, space="PSUM") as psbc,
            tc.tile_pool(name="pstr", bufs=2, space="PSUM") as pstr,
            tc.tile_pool(name="pssm", bufs=2, space="PSUM") as pssm,
        ):
            # ---------------- constants ----------------
            # The PE program is PURE bf16 (matmuls and transposes): mixing
            # PE dtype modes (bf16 matmuls interleaved with f32r
            # transposes) faults the exec unit on hardware.
            ident_f = cpool.tile([P, P], F32)
            make_identity(nc, ident_f)
            ident = cpool.tile([P, P], BF16)
            nc.scalar.copy(ident[:], ident_f[:])
            ones_col = cpool.tile([P, 1], BF16)    # partition-sum lhsT
            nc.vector.memset(ones_col[:], 1.0)
            ones_rowb = cpool.tile([1, P], BF16)   # partition-broadcast lhsT
            nc.vector.memset(ones_rowb[:], 1.0)

            # ---------------- small loads (ACT ring) ----------------
            w1c = wpool.tile([P, KT], BF16, tag="w1c")
            nc.scalar.dma_start(w1c[:], w1c_d[:])
            bi2c = wpool.tile([P, KT], F32, tag="bi2c")
            nc.scalar.dma_start(bi2c[:], bi2c_d[:])
            bm2c = wpool.tile([P, KT], F32, tag="bm2c")
            nc.scalar.dma_start(bm2c[:], bm2c_d[:])
            mask_col = wpool.tile([P, MT], F32, tag="maskc")
            nc.scalar.dma_start(mask_col[:], mask_d.rearrange("1 (o p) -> p o", p=P))
            wm1_bc = wpool.tile([P, HID], BF16, tag="wm1bc")
            nc.scalar.dma_start(wm1_bc[:], wm1_d.to_broadcast([P, HID]))

            # ---------------- big reads ----------------
            # W_in2^T halves (oc 0-3 / 4-7) on ACT ring; input^T l-groups
            # on the Pool(SWDGE) ring so neither sequencer drowns in DMA
            # config time. Arrival order matches the mm loop's needs.
            w2th = {}
            for h in range(2):
                for k in range(KT):
                    t = wpool.tile([P, GL], BF16, tag=f"w2t{k}_{h}")
                    nc.scalar.dma_start(
                        t[:], w2t_d[k * P:(k + 1) * P, h * GL:(h + 1) * GL]
                    )
                    w2th[(k, h)] = t
            inTg = {}
            for g in range(GT):
                for k in range(KT):
                    t = inpool.tile([P, GL], BF16, tag=f"inT{k}_{g}")
                    nc.sync.dma_start(
                        t[:], inT_d[k * P:(k + 1) * P, g * GL:(g + 1) * GL]
                    )
                    inTg[(k, g)] = t
            mem_t = wpool.tile([P, MT, HID], BF16, tag="memt")
            nc.sync.dma_start(mem_t[:], mem_d.rearrange("(j p) d -> p j d", p=P))
            wm2t = {}
            for k in range(KT):
                t = wpool.tile([P, HID], BF16, tag=f"wm2t{k}")
                nc.scalar.dma_start(t[:], wm2t_d[k * P:(k + 1) * P, :])
                wm2t[k] = t

            # ---------------- persistent state ----------------
            i2T = [i2pool.tile([P, LD], BF16, tag=f"i2_{oc}", name=f"i2_{oc}")
                   for oc in range(KT)]
            id_col = smpool.tile([P, LT], BF16, tag="idcol")
            id_row = smpool.tile([1, LD], BF16, tag="idrow")
            e_bc = smpool.tile([P, LD], BF16, tag="ebc")
            zacc = smpool.tile([P, GT], F32, tag="zacc")
            z_col = smpool.tile([P, 1], F32, tag="zcol")
            rz_col = smpool.tile([P, 1], F32, tag="rzcol")
            q_part = smpool.tile([P, KT, GT], F32, tag="qpart")
            q_col = smpool.tile([P, KT], F32, tag="qcol")
            q_colb = smpool.tile([P, KT], BF16, tag="qcolb")
            s_mcol = smpool.tile([P, MT], F32, tag="smcol")
            msk = smpool.tile([P, MT], F32, tag="msk")
            e_s = smpool.tile([P, MT], BF16, tag="es")
            es_r = smpool.tile([P, 1], F32, tag="esr")
            es_rb = smpool.tile([P, 1], BF16, tag="esrb")
            zs_sb = smpool.tile([1, 1], BF16, tag="zssb")
            zsbc_sb = smpool.tile([P, 1], F32, tag="zsbc")
            rzs_col = smpool.tile([P, 1], F32, tag="rzs")
            p_col = smpool.tile([P, KT], BF16, tag="pcol")
            v_colf = smpool.tile([P, KT], F32, tag="vcolf")
            v_col = smpool.tile([P, KT], F32, tag="vcol")
            v_colb = smpool.tile([P, KT], BF16, tag="vcolb")
            sq_col = smpool.tile([P, KT], F32, tag="sqcol")
            u_col = smpool.tile([P, KT], F32, tag="ucol")

            # ---------------- emitters ----------------
            def emit_mm(g, oc):
                ps = psmm.tile([P, GL], F32, tag="mm", name=f"mm{g}_{oc}")
                for k in range(KT):
                    nc.tensor.matmul(
                        ps[:],
                        w2th[(k, oc // 4)][:, (oc % 4) * P:(oc % 4 + 1) * P],
                        inTg[(k, g)][:],
                        start=(k == 0), stop=(k == KT - 1),
                    )
                nc.scalar.activation(
                    i2T[oc][:, g * GL:(g + 1) * GL], ps[:], IDENT,
                    bias=bi2c[:, oc:oc + 1],
                )
                nc.sync.dma_start(
                    o1T_d[oc * P:(oc + 1) * P, g * GL:(g + 1) * GL],
                    i2T[oc][:, g * GL:(g + 1) * GL],
                )

            def emit_idot(i):
                g, ii = divmod(i, GT)
                ps = pssm.tile([P, 1], F32, tag="sm", name=f"id{i}")
                for k in range(KT):
                    nc.tensor.matmul(
                        ps[:], inTg[(k, g)][:, ii * P:(ii + 1) * P],
                        w1c[:, k:k + 1],
                        start=(k == 0), stop=(k == KT - 1),
                    )
                nc.scalar.copy(id_col[:, i:i + 1], ps[:])

            def emit_tr(i):
                ps = pstr.tile([1, P], BF16, tag="tr", name=f"tr{i}")
                nc.tensor.transpose(ps[:], id_col[:, i:i + 1], ident)
                nc.scalar.copy(id_row[:, i * P:(i + 1) * P], ps[:])

            def emit_ebc(g):
                ps = psbc.tile([P, GL], F32, tag="bc", name=f"ebc{g}")
                nc.tensor.matmul(
                    ps[:], ones_rowb[:], id_row[:, g * GL:(g + 1) * GL],
                    start=True, stop=True,
                )
                nc.scalar.activation(
                    e_bc[:, g * GL:(g + 1) * GL], ps[:], EXP,
                    accum_out=zacc[:, g:g + 1],
                )

            def emit_q(g):
                # q_part[d, k, g] = sum_{l in g} inT[d,l] * e[l]
                for k in range(KT):
                    scr = scrpool.tile([P, GL], BF16, tag="scr", name=f"q{k}_{g}")
                    nc.vector.tensor_mul(
                        scr[:], inTg[(k, g)][:], e_bc[:, g * GL:(g + 1) * GL]
                    )
                    nc.vector.tensor_reduce(
                        q_part[:, k, g:g + 1], scr[:], AX.X, OP.add
                    )

            def emit_smul(j):
                scrm = scrpool.tile([P, HID], BF16, tag="scrm", name=f"smul{j}")
                nc.vector.tensor_mul(scrm[:], mem_t[:, j, :], wm1_bc[:])
                nc.vector.tensor_reduce(
                    s_mcol[:, j:j + 1], scrm[:], AX.X, OP.add
                )

            def emit_es():
                nc.vector.tensor_scalar(msk[:], mask_col[:], -1.0, 1e30,
                                        OP.add, OP.mult)
                nc.vector.tensor_add(msk[:], msk[:], s_mcol[:])
                nc.scalar.activation(e_s[:], msk[:], EXP)

            def emit_zs():
                # Z_s = sum over all m of e_s; broadcast 1/Z_s to a column
                nc.vector.tensor_reduce(es_r[:], e_s[:], AX.X, OP.add)
                nc.scalar.copy(es_rb[:], es_r[:])
                ps = pssm.tile([P, 1], F32, tag="sm", name="zs")
                nc.tensor.matmul(ps[0:1, :], ones_col[:], es_rb[:],
                                 start=True, stop=True)
                nc.scalar.copy(zs_sb[:], ps[0:1, :])
                ps2 = pssm.tile([P, 1], F32, tag="sm", name="zsbc")
                nc.tensor.matmul(ps2[:], ones_rowb[:], zs_sb[:],
                                 start=True, stop=True)
                nc.scalar.copy(zsbc_sb[:], ps2[:])
                nc.vector.reciprocal(rzs_col[:], zsbc_sb[:])

            def emit_p(dc):
                ps = pssm.tile([P, 1], F32, tag="sm", name=f"p{dc}")
                for j in range(MT):
                    nc.tensor.matmul(
                        ps[:], mem_t[:, j, dc * P:(dc + 1) * P],
                        e_s[:, j:j + 1],
                        start=(j == 0), stop=(j == MT - 1),
                    )
                nc.scalar.activation(p_col[:, dc:dc + 1], ps[:], COPY,
                                     scale=rzs_col[:, 0:1])

            def emit_v(oc):
                ps = pssm.tile([P, 1], F32, tag="sm", name=f"v{oc}")
                for k in range(KT):
                    nc.tensor.matmul(
                        ps[:], wm2t[k][:, oc * P:(oc + 1) * P],
                        p_col[:, k:k + 1],
                        start=(k == 0), stop=(k == KT - 1),
                    )
                nc.scalar.copy(v_colf[:, oc:oc + 1], ps[:])

            def emit_vfin():
                nc.vector.tensor_add(v_col[:], v_colf[:], bm2c[:])
                nc.vector.tensor_copy(v_colb[:], v_col[:])
                nc.sync.dma_start(vrow_d[:], v_col[:])

            def emit_qfin():
                for k in range(KT):
                    nc.vector.tensor_reduce(
                        q_col[:, k:k + 1], q_part[:, k, :], AX.X, OP.add
                    )
                nc.vector.tensor_copy(q_colb[:], q_col[:])
                nc.vector.tensor_reduce(z_col[:], zacc[:], AX.X, OP.add)
                nc.vector.reciprocal(rz_col[:], z_col[:])

            def emit_s(oc):
                ps = pssm.tile([P, 1], F32, tag="sm", name=f"s{oc}")
                for k in range(KT):
                    nc.tensor.matmul(
                        ps[:],
                        w2th[(k, oc // 4)][:, (oc % 4) * P:(oc % 4 + 1) * P],
                        q_colb[:, k:k + 1],
                        start=(k == 0), stop=(k == KT - 1),
                    )
                nc.scalar.copy(sq_col[:, oc:oc + 1], ps[:])

            def emit_u():
                nc.vector.tensor_scalar(u_col[:], bi2c[:], z_col[:, 0:1],
                                        None, OP.mult)
                nc.vector.tensor_add(u_col[:], u_col[:], sq_col[:])
                nc.vector.tensor_scalar(u_col[:], u_col[:], rz_col[:, 0:1],
                                        None, OP.mult)
                nc.vector.tensor_mul(u_col[:], u_col[:], v_col[:])
                nc.sync.dma_start(urow_d[:], u_col[:])

            def emit_prod(g, oc):
                o3 = o3pool.tile([P, GL], BF16, tag="o3", name=f"o3_{g}_{oc}")
                nc.vector.tensor_scalar(
                    o3[:], i2T[oc][:, g * GL:(g + 1) * GL],
                    v_col[:, oc:oc + 1], None, OP.mult,
                )
                nc.sync.dma_start(
                    o3T_d[oc * P:(oc + 1) * P, g * GL:(g + 1) * GL], o3[:]
                )

            # ---------------- schedule ----------------
            # mm tiles t=0..31 (g outer, oc inner); events spliced between
            # tiles at points where their inputs are guaranteed resident,
            # keeping every engine's in-order stream stall-free.
            def splice(t):
                if _LEVEL >= 1 and 1 <= t <= LT:
                    emit_idot(t - 1)
                if _LEVEL >= 2 and 4 <= t <= LT + 3:
                    emit_tr(t - 4)
                if _LEVEL >= 2 and t in (8, 12, 16, 20):
                    emit_ebc((t - 8) // 4)
                if _LEVEL >= 3 and t in (9, 13, 17, 21):
                    emit_q((t - 9) // 4)
                if _LEVEL >= 4 and t == 14:
                    for j in range(MT):
                        emit_smul(j)
                if _LEVEL >= 4 and t == 15:
                    emit_es()
                if _LEVEL >= 4 and t == 16:
                    emit_zs()
                if _LEVEL >= 5 and t in (17, 18):
                    for dc in range(4):
                        emit_p((t - 17) * 4 + dc)
                if _LEVEL >= 5 and t in (19, 20):
                    for oc in range(4):
                        emit_v((t - 19) * 4 + oc)
                if _LEVEL >= 5 and t == 21:
                    emit_vfin()
                if t == 22:
                    if _LEVEL >= 3:
                        emit_qfin()
                    if _LEVEL >= 7:
                        for oc in range(KT):
                            emit_prod(0, oc)
                if _LEVEL >= 7 and t == 23:
                    for oc in range(KT):
                        emit_prod(1, oc)
                # prod(2, *) must wait until mm(2, 7) has been emitted (t=24)
                if _LEVEL >= 7 and t == 24:
                    for oc in range(KT):
                        emit_prod(2, oc)
                if _LEVEL >= 6 and t in (24, 25):
                    for oc in range(4):
                        emit_s((t - 24) * 4 + oc)
                if _LEVEL >= 6 and t == 26:
                    emit_u()

            t = 0
            for g in range(GT):
                for oc in range(KT):
                    emit_mm(g, oc)
                    if _LEVEL >= 7 and g == GT - 1:
                        emit_prod(g, oc)
                    t += 1
                    splice(t)

    nc.finalize()
    return nc


def _get_nc():
    global _NC_CACHE
    if _NC_CACHE is None:
        _NC_CACHE = _build_nc()
    return _NC_CACHE


def kernel(**inputs) -> np.ndarray:
    nc = _get_nc()
    bf16 = ml_dtypes.bfloat16

    inp = np.asarray(inputs["input"], np.float32)
    mem = np.asarray(inputs["memory"], np.float32)
    mask = np.asarray(inputs["mask"], np.float32)
    w_in1 = np.asarray(inputs["w_in1"], np.float32).reshape(HID)
    w_mem1 = np.asarray(inputs["w_mem1"], np.float32).reshape(1, HID)
    W_in2 = np.asarray(inputs["W_in2"], np.float32)
    b_in2 = np.asarray(inputs["b_in2"], np.float32).reshape(HID)
    W_mem2 = np.asarray(inputs["W_mem2"], np.float32)
    b_mem2 = np.asarray(inputs["b_mem2"], np.float32).reshape(HID)

    w2t = W_in2.T.astype(bf16)
    wm2t = W_mem2.T.astype(bf16)
    w1c = w_in1.reshape(KT, P).T.astype(bf16)
    wm1 = w_mem1.astype(bf16)
    bi2c = np.ascontiguousarray(b_in2.reshape(KT, P).T)
    bm2c = np.ascontiguousarray(b_mem2.reshape(KT, P).T)

    in_maps = []
    for b in range(N_CORES):
        in_maps.append({
            "inT": inp[b].T.astype(bf16),
            "w2t": w2t,
            "wm2t": wm2t,
            "memory": mem[b].astype(bf16),
            "mask": np.ascontiguousarray(mask[b].reshape(1, LM)),
            "w1c": w1c,
            "wm1": wm1,
            "bi2c": bi2c,
            "bm2c": bm2c,
        })

    res = run_bass_kernel_spmd(nc, in_maps, core_ids=list(range(N_CORES)))

    out = np.empty((BSZ, 4 * LD, HID), np.float32)
    for b in range(N_CORES):
        r = res.results[b]
        out[b, 0:LD] = r["o1T"].T
        v = r["vrow"].T.reshape(HID).astype(np.float32)
        out[b, LD:2 * LD] = v
        out[b, 2 * LD:3 * LD] = r["o3T"].T
        u = r["urow"].T.reshape(HID).astype(np.float32)
        out[b, 3 * LD:4 * LD] = u
    return out


# revision 34
# speedup vs baseline: 1.6773x; 1.1447x over previous
"""Trainium2 Bass kernel for nn_BiAttention (sparse_attention).

Math: the attention matrix is rank-1 plus a mask bias:
    att[b,l,m] = idot[b,l] + s_m[b,m]
Row softmax over m is invariant to the per-row constant idot[b,l], so
    output_one[b,l,:] = v_b := softmax_m(s_m) @ (memory @ W_mem2.T + b_mem2)
and max_m att = idot + const, so
    output_two[b,0,:] = softmax_l(idot) @ inp2 = (W_in2 @ q + Z*b_in2)/Z
    with q[d] = sum_l e_l * input[l,d],  Z = sum_l e_l,  e = exp(idot)
Output row blocks [N, 4*Ld, d]:
    [0:2048]    inp2 = input @ W_in2.T + b_in2        (full rank, device)
    [2048:4096] v_b broadcast                          (host replicates row)
    [4096:6144] inp2 * v_b                             (full rank, device)
    [6144:8192] (output_two * v_b) broadcast           (host replicates row)

Device computes everything TRANSPOSED (output features on partitions):
    i2T[o, l] = inp2[l, o] = sum_d W_in2^T[d,o] * input^T[d,l] + b[o]
so the bias is a per-partition scalar (ACT engine Identity+bias) and
prod = i2T * v[o] is a per-partition tensor_scalar on DVE. All matmul
operands are bf16 (inputs/weights pre-transposed + converted on host),
outputs stored bf16 and upconverted on host: rel err ~5e-3 << 2e-2.

Sharding: pure data parallel, one batch element per NeuronCore (8 cores).

Engine budget @2.4GHz PE: main matmul 131k cyc; everything else rides in
N=1 matmul columns (idot/p/v/s ~300 matmuls of ~1 cycle) or on DVE/ACT.
DMA: reads 9.5MB + writes 8.4MB ~= 50us < PE ~60us. Reads split across
ACT (weights) and Pool/SWDGE (input) rings; writes on SP ring.
"""

import numpy as np
import ml_dtypes

import concourse.bass as bass
import concourse.tile as tile
from concourse import bacc, mybir
from concourse.bass_utils import run_bass_kernel_spmd
from concourse.masks import make_identity

F32 = mybir.dt.float32
F32R = mybir.dt.float32r
BF16 = mybir.dt.bfloat16
AX = mybir.AxisListType
OP = mybir.AluOpType
EXP = mybir.ActivationFunctionType.Exp
IDENT = mybir.ActivationFunctionType.Identity
COPY = mybir.ActivationFunctionType.Copy

P = 128
BSZ, LD, LM, HID = 8, 2048, 512, 1024
KT = HID // P          # 8 hidden-dim chunks
LT = LD // P           # 16 l slices of 128
MT = LM // P           # 4 memory tiles
GT = 4                 # l groups of 512
GL = LD // GT          # 512
N_CORES = 8

_NC_CACHE = None

# bisection gate: 0=mm only, 1=+idot, 2=+tr/ebc, 3=+q, 4=+mem path,
# 5=+p/v, 6=+s/u, 7=full (prod)
import os as _os
_LEVEL = int(_os.environ.get("KLEVEL", "7"))


def _build_nc():
    nc = bacc.Bacc("TRN2", target_bir_lowering=False, num_devices=N_CORES)

    inT_d = nc.dram_tensor("inT", [HID, LD], BF16, kind="ExternalInput").ap()
    w2t_d = nc.dram_tensor("w2t", [HID, HID], BF16, kind="ExternalInput").ap()
    wm2t_d = nc.dram_tensor("wm2t", [HID, HID], BF16, kind="ExternalInput").ap()
    mem_d = nc.dram_tensor("memory", [LM, HID], BF16, kind="ExternalInput").ap()
    mask_d = nc.dram_tensor("mask", [1, LM], F32, kind="ExternalInput").ap()
    w1c_d = nc.dram_tensor("w1c", [P, KT], BF16, kind="ExternalInput").ap()
    wm1_d = nc.dram_tensor("wm1", [1, HID], BF16, kind="ExternalInput").ap()
    bi2c_d = nc.dram_tensor("bi2c", [P, KT], F32, kind="ExternalInput").ap()
    bm2c_d = nc.dram_tensor("bm2c", [P, KT], F32, kind="ExternalInput").ap()
    o1T_d = nc.dram_tensor("o1T", [HID, LD], BF16, kind="ExternalOutput").ap()
    o3T_d = nc.dram_tensor("o3T", [HID, LD], BF16, kind="ExternalOutput").ap()
    vrow_d = nc.dram_tensor("vrow", [P, KT], F32, kind="ExternalOutput").ap()
    urow_d = nc.dram_tensor("urow", [P, KT], F32, kind="ExternalOutput").ap()

    with tile.TileContext(nc) as tc:
        with (
            tc.tile_pool(name="const", bufs=1) as cpool,
            tc.tile_pool(name="w", bufs=1) as wpool,
            tc.tile_pool(name="inp", bufs=1) as inpool,
            tc.tile_pool(name="i2", bufs=1) as i2pool,
            tc.tile_pool(name="o3", bufs=1) as o3pool,
            tc.tile_pool(name="scr", bufs=2) as scrpool,
            tc.tile_pool(name="sm", bufs=1) as smpool,
            tc.tile_pool(name="psmm", bufs=3, space="PSUM") as psmm,
            tc.tile_pool(name="psbc", bufs=1, space="PSUM") as psbc,
            tc.tile_pool(name="pstr", bufs=2, space="PSUM") as pstr,
            tc.tile_pool(name="pssm", bufs=2, space="PSUM") as pssm,
        ):
            # ---------------- constants ----------------
            # The PE program is PURE bf16 (matmuls and transposes): mixing
            # PE dtype modes (bf16 matmuls interleaved with f32r
            # transposes) faults the exec unit on hardware.
            ident_f = cpool.tile([P, P], F32)
            make_identity(nc, ident_f)
            ident = cpool.tile([P, P], BF16)
            nc.scalar.copy(ident[:], ident_f[:])
            ones_col = cpool.tile([P, 1], BF16)    # partition-sum lhsT
            nc.vector.memset(ones_col[:], 1.0)
            ones_rowb = cpool.tile([1, P], BF16)   # partition-broadcast lhsT
            nc.vector.memset(ones_rowb[:], 1.0)

            # ---------------- small loads (ACT ring) ----------------
            w1c = wpool.tile([P, KT], BF16, tag="w1c")
            nc.scalar.dma_start(w1c[:], w1c_d[:])
            bi2c = wpool.tile([P, KT], F32, tag="bi2c")
            nc.scalar.dma_start(bi2c[:], bi2c_d[:])
            bm2c = wpool.tile([P, KT], F32, tag="bm2c")
            nc.scalar.dma_start(bm2c[:], bm2c_d[:])
            mask_col = wpool.tile([P, MT], F32, tag="maskc")
            nc.scalar.dma_start(mask_col[:], mask_d.rearrange("1 (o p) -> p o", p=P))
            wm1_bc = wpool.tile([P, HID], BF16, tag="wm1bc")
            nc.scalar.dma_start(wm1_bc[:], wm1_d.to_broadcast([P, HID]))

            # ---------------- big reads (all ACT ring) ----------------
            # Consolidated into 8 large DMAs so the sequencers don't drown
            # in per-DMA config time, ordered by when the mm loop needs the
            # data. Writes own the SP ring exclusively.
            w2th_sb = []
            inTg_sb = []

            def load_w2th(h):
                w = wpool.tile([P, KT, GL], BF16, tag=f"w2th{h}",
                               name=f"w2th{h}")
                nc.scalar.dma_start(
                    w[:],
                    w2t_d[:, h * GL:(h + 1) * GL].rearrange(
                        "(k p) x -> p k x", p=P),
                )
                w2th_sb.append(w)

            def load_inTg(g):
                t = inpool.tile([P, KT, GL], BF16, tag=f"inTg{g}",
                                name=f"inTg{g}")
                nc.scalar.dma_start(
                    t[:],
                    inT_d[:, g * GL:(g + 1) * GL].rearrange(
                        "(k p) x -> p k x", p=P),
                )
                inTg_sb.append(t)

            load_w2th(0)
            load_inTg(0)
            load_w2th(1)
            load_inTg(1)
            load_inTg(2)
            load_inTg(3)
            mem_t = wpool.tile([P, MT, HID], BF16, tag="memt")
            nc.scalar.dma_start(mem_t[:], mem_d.rearrange("(j p) d -> p j d", p=P))
            wm2t_sb = wpool.tile([P, KT, HID], BF16, tag="wm2t")
            nc.scalar.dma_start(
                wm2t_sb[:], wm2t_d.rearrange("(k p) d -> p k d", p=P)
            )

            # ---------------- persistent state ----------------
            i2T = [i2pool.tile([P, LD], BF16, tag=f"i2_{oc}", name=f"i2_{oc}")
                   for oc in range(KT)]
            id_col = smpool.tile([P, LT], BF16, tag="idcol")
            id_row = smpool.tile([1, LD], BF16, tag="idrow")
            e_bc = smpool.tile([P, LD], BF16, tag="ebc")
            zacc = smpool.tile([P, GT], F32, tag="zacc")
            z_col = smpool.tile([P, 1], F32, tag="zcol")
            rz_col = smpool.tile([P, 1], F32, tag="rzcol")
            q_part = smpool.tile([P, KT, GT], F32, tag="qpart")
            q_col = smpool.tile([P, KT], F32, tag="qcol")
            q_colb = smpool.tile([P, KT], BF16, tag="qcolb")
            s_mcol = smpool.tile([P, MT], F32, tag="smcol")
            msk = smpool.tile([P, MT], F32, tag="msk")
            e_s = smpool.tile([P, MT], BF16, tag="es")
            es_r = smpool.tile([P, 1], F32, tag="esr")
            es_rb = smpool.tile([P, 1], BF16, tag="esrb")
            zs_sb = smpool.tile([1, 1], BF16, tag="zssb")
            zsbc_sb = smpool.tile([P, 1], F32, tag="zsbc")
            rzs_col = smpool.tile([P, 1], F32, tag="rzs")
            p_col = smpool.tile([P, KT], BF16, tag="pcol")
            v_colf = smpool.tile([P, KT], F32, tag="vcolf")
            v_col = smpool.tile([P, KT], F32, tag="vcol")
            sq_col = smpool.tile([P, KT], F32, tag="sqcol")
            u_col = smpool.tile([P, KT], F32, tag="ucol")
            o3sb = [o3pool.tile([P, LD], BF16, tag=f"o3_{oc}", name=f"o3_{oc}")
                    for oc in range(KT)]

            # ---------------- emitters ----------------
            def emit_mm(g, oc):
                ps = psmm.tile([P, GL], F32, tag="mm", name=f"mm{g}_{oc}")
                for k in range(KT):
                    nc.tensor.matmul(
                        ps[:],
                        w2th_sb[oc // 4][:, k, (oc % 4) * P:(oc % 4 + 1) * P],
                        inTg_sb[g][:, k, :],
                        start=(k == 0), stop=(k == KT - 1),
                    )
                nc.scalar.activation(
                    i2T[oc][:, g * GL:(g + 1) * GL], ps[:], IDENT,
                    bias=bi2c[:, oc:oc + 1],
                )
                nc.sync.dma_start(
                    o1T_d[oc * P:(oc + 1) * P, g * GL:(g + 1) * GL],
                    i2T[oc][:, g * GL:(g + 1) * GL],
                )

            def emit_idot(i):
                g, ii = divmod(i, GT)
                ps = pssm.tile([P, 1], F32, tag="sm", name=f"id{i}")
                for k in range(KT):
                    nc.tensor.matmul(
                        ps[:], inTg_sb[g][:, k, ii * P:(ii + 1) * P],
                        w1c[:, k:k + 1],
                        start=(k == 0), stop=(k == KT - 1),
                    )
                nc.scalar.copy(id_col[:, i:i + 1], ps[:])

            def emit_tr(i):
                ps = pstr.tile([1, P], BF16, tag="tr", name=f"tr{i}")
                nc.tensor.transpose(ps[:], id_col[:, i:i + 1], ident)
                nc.scalar.copy(id_row[:, i * P:(i + 1) * P], ps[:])

            def emit_ebc(g):
                ps = psbc.tile([P, GL], F32, tag="bc", name=f"ebc{g}")
                nc.tensor.matmul(
                    ps[:], ones_rowb[:], id_row[:, g * GL:(g + 1) * GL],
                    start=True, stop=True,
                )
                nc.scalar.activation(
                    e_bc[:, g * GL:(g + 1) * GL], ps[:], EXP,
                    accum_out=zacc[:, g:g + 1],
                )

            def emit_q(g):
                # q_part[d, k, g] = sum_{l in g} inT[d,l] * e[l]
                for k in range(KT):
                    scr = scrpool.tile([P, GL], BF16, tag="scr", name=f"q{k}_{g}")
                    nc.vector.tensor_mul(
                        scr[:], inTg_sb[g][:, k, :], e_bc[:, g * GL:(g + 1) * GL]
                    )
                    nc.vector.tensor_reduce(
                        q_part[:, k, g:g + 1], scr[:], AX.X, OP.add
                    )

            def emit_smul(j):
                scrm = scrpool.tile([P, HID], BF16, tag="scrm", name=f"smul{j}")
                nc.vector.tensor_mul(scrm[:], mem_t[:, j, :], wm1_bc[:])
                nc.vector.tensor_reduce(
                    s_mcol[:, j:j + 1], scrm[:], AX.X, OP.add
                )

            def emit_es():
                nc.vector.tensor_scalar(msk[:], mask_col[:], -1.0, 1e30,
                                        OP.add, OP.mult)
                nc.vector.tensor_add(msk[:], msk[:], s_mcol[:])
                nc.scalar.activation(e_s[:], msk[:], EXP)

            def emit_zs():
                # Z_s = sum over all m of e_s; broadcast 1/Z_s to a column
                nc.vector.tensor_reduce(es_r[:], e_s[:], AX.X, OP.add)
                nc.scalar.copy(es_rb[:], es_r[:])
                ps = pssm.tile([P, 1], F32, tag="sm", name="zs")
                nc.tensor.matmul(ps[0:1, :], ones_col[:], es_rb[:],
                                 start=True, stop=True)
                nc.scalar.copy(zs_sb[:], ps[0:1, :])
                ps2 = pssm.tile([P, 1], F32, tag="sm", name="zsbc")
                nc.tensor.matmul(ps2[:], ones_rowb[:], zs_sb[:],
                                 start=True, stop=True)
                nc.scalar.copy(zsbc_sb[:], ps2[:])
                nc.vector.reciprocal(rzs_col[:], zsbc_sb[:])

            def emit_p(dc):
                ps = pssm.tile([P, 1], F32, tag="sm", name=f"p{dc}")
                for j in range(MT):
                    nc.tensor.matmul(
                        ps[:], mem_t[:, j, dc * P:(dc + 1) * P],
                        e_s[:, j:j + 1],
                        start=(j == 0), stop=(j == MT - 1),
                    )
                nc.scalar.activation(p_col[:, dc:dc + 1], ps[:], COPY,
                                     scale=rzs_col[:, 0:1])

            def emit_v(oc):
                ps = pssm.tile([P, 1], F32, tag="sm", name=f"v{oc}")
                for k in range(KT):
                    nc.tensor.matmul(
                        ps[:], wm2t_sb[:, k, oc * P:(oc + 1) * P],
                        p_col[:, k:k + 1],
                        start=(k == 0), stop=(k == KT - 1),
                    )
                nc.scalar.copy(v_colf[:, oc:oc + 1], ps[:])

            def emit_vfin():
                nc.vector.tensor_add(v_col[:], v_colf[:], bm2c[:])
                nc.sync.dma_start(vrow_d[:], v_col[:])

            def emit_qfin():
                for k in range(KT):
                    nc.vector.tensor_reduce(
                        q_col[:, k:k + 1], q_part[:, k, :], AX.X, OP.add
                    )
                nc.vector.tensor_copy(q_colb[:], q_col[:])
                nc.vector.tensor_reduce(z_col[:], zacc[:], AX.X, OP.add)
                nc.vector.reciprocal(rz_col[:], z_col[:])

            def emit_s(oc):
                ps = pssm.tile([P, 1], F32, tag="sm", name=f"s{oc}")
                for k in range(KT):
                    nc.tensor.matmul(
                        ps[:],
                        w2th_sb[oc // 4][:, k, (oc % 4) * P:(oc % 4 + 1) * P],
                        q_colb[:, k:k + 1],
                        start=(k == 0), stop=(k == KT - 1),
                    )
                nc.scalar.copy(sq_col[:, oc:oc + 1], ps[:])

            def emit_u():
                nc.vector.tensor_scalar(u_col[:], bi2c[:], z_col[:, 0:1],
                                        None, OP.mult)
                nc.vector.tensor_add(u_col[:], u_col[:], sq_col[:])
                nc.vector.tensor_scalar(u_col[:], u_col[:], rz_col[:, 0:1],
                                        None, OP.mult)
                nc.vector.tensor_mul(u_col[:], u_col[:], v_col[:])
                nc.sync.dma_start(urow_d[:], u_col[:])

            def emit_prod(g, oc):
                nc.vector.tensor_scalar(
                    o3sb[oc][:, g * GL:(g + 1) * GL],
                    i2T[oc][:, g * GL:(g + 1) * GL],
                    v_col[:, oc:oc + 1], None, OP.mult,
                )
                if g == 2:
                    nc.sync.dma_start(
                        o3T_d[oc * P:(oc + 1) * P, 0:3 * GL],
                        o3sb[oc][:, 0:3 * GL],
                    )
                elif g == 3:
                    nc.sync.dma_start(
                        o3T_d[oc * P:(oc + 1) * P, 3 * GL:LD],
                        o3sb[oc][:, 3 * GL:LD],
                    )

            # ---------------- schedule ----------------
            # mm tiles t=0..31 (g outer, oc inner); events spliced between
            # tiles at points where their inputs are guaranteed resident,
            # keeping every engine's in-order stream stall-free.
            def splice(t):
                if _LEVEL >= 1 and 1 <= t <= LT:
                    emit_idot(t - 1)
                if _LEVEL >= 2 and 4 <= t <= LT + 3:
                    emit_tr(t - 4)
                if _LEVEL >= 2 and t in (8, 12, 16, 20):
                    emit_ebc((t - 8) // 4)
                if _LEVEL >= 3 and t in (9, 13, 17, 21):
                    emit_q((t - 9) // 4)
                if _LEVEL >= 4 and t == 14:
                    for j in range(MT):
                        emit_smul(j)
                if _LEVEL >= 4 and t == 15:
                    emit_es()
                if _LEVEL >= 4 and t == 16:
                    emit_zs()
                if _LEVEL >= 5 and t in (17, 18):
                    for dc in range(4):
                        emit_p((t - 17) * 4 + dc)
                if _LEVEL >= 5 and t in (19, 20):
                    for oc in range(4):
                        emit_v((t - 19) * 4 + oc)
                if _LEVEL >= 5 and t == 21:
                    emit_vfin()
                if t == 22:
                    if _LEVEL >= 3:
                        emit_qfin()
                    if _LEVEL >= 7:
                        for oc in range(KT):
                            emit_prod(0, oc)
                if _LEVEL >= 7 and t == 23:
                    for oc in range(KT):
                        emit_prod(1, oc)
                # prod(2, *) must wait until mm(2, 7) has been emitted (t=24)
                if _LEVEL >= 7 and t == 24:
                    for oc in range(KT):
                        emit_prod(2, oc)
                if _LEVEL >= 6 and t in (23, 24):
                    for oc in range(4):
                        emit_s((t - 23) * 4 + oc)
                if _LEVEL >= 6 and t == 25:
                    emit_u()

            t = 0
            for g in range(GT):
                for oc in range(KT):
                    emit_mm(g, oc)
                    if _LEVEL >= 7 and g == GT - 1:
                        emit_prod(g, oc)
                    t += 1
                    splice(t)

    nc.finalize()
    return nc


def _get_nc():
    global _NC_CACHE
    if _NC_CACHE is None:
        _NC_CACHE = _build_nc()
    return _NC_CACHE


def kernel(**inputs) -> np.ndarray:
    nc = _get_nc()
    bf16 = ml_dtypes.bfloat16

    inp = np.asarray(inputs["input"], np.float32)
    mem = np.asarray(inputs["memory"], np.float32)
    mask = np.asarray(inputs["mask"], np.float32)
    w_in1 = np.asarray(inputs["w_in1"], np.float32).reshape(HID)
    w_mem1 = np.asarray(inputs["w_mem1"], np.float32).reshape(1, HID)
    W_in2 = np.asarray(inputs["W_in2"], np.float32)
    b_in2 = np.asarray(inputs["b_in2"], np.float32).reshape(HID)
    W_mem2 = np.asarray(inputs["W_mem2"], np.float32)
    b_mem2 = np.asarray(inputs["b_mem2"], np.float32).reshape(HID)

    w2t = W_in2.T.astype(bf16)
    wm2t = W_mem2.T.astype(bf16)
    w1c = w_in1.reshape(KT, P).T.astype(bf16)
    wm1 = w_mem1.astype(bf16)
    bi2c = np.ascontiguousarray(b_in2.reshape(KT, P).T)
    bm2c = np.ascontiguousarray(b_mem2.reshape(KT, P).T)

    in_maps = []
    for b in range(N_CORES):
        in_maps.append({
            "inT": inp[b].T.astype(bf16),
            "w2t": w2t,
            "wm2t": wm2t,
            "memory": mem[b].astype(bf16),
            "mask": np.ascontiguousarray(mask[b].reshape(1, LM)),
            "w1c": w1c,
            "wm1": wm1,
            "bi2c": bi2c,
            "bm2c": bm2c,
        })

    res = run_bass_kernel_spmd(nc, in_maps, core_ids=list(range(N_CORES)))

    out = np.empty((BSZ, 4 * LD, HID), np.float32)
    for b in range(N_CORES):
        r = res.results[b]
        out[b, 0:LD] = r["o1T"].T
        v = r["vrow"].T.reshape(HID).astype(np.float32)
        out[b, LD:2 * LD] = v
        out[b, 2 * LD:3 * LD] = r["o3T"].T
        u = r["urow"].T.reshape(HID).astype(np.float32)
        out[b, 3 * LD:4 * LD] = u
    return out


# revision 35
# speedup vs baseline: 1.7720x; 1.0564x over previous
"""Trainium2 Bass kernel for nn_BiAttention (sparse_attention).

Math: the attention matrix is rank-1 plus a mask bias:
    att[b,l,m] = idot[b,l] + s_m[b,m]
Row softmax over m is invariant to the per-row constant idot[b,l], so
    output_one[b,l,:] = v_b := softmax_m(s_m) @ (memory @ W_mem2.T + b_mem2)
and max_m att = idot + const, so
    output_two[b,0,:] = softmax_l(idot) @ inp2 = (W_in2 @ q + Z*b_in2)/Z
    with q[d] = sum_l e_l * input[l,d],  Z = sum_l e_l,  e = exp(idot)
Output row blocks [N, 4*Ld, d]:
    [0:2048]    inp2 = input @ W_in2.T + b_in2        (full rank, device)
    [2048:4096] v_b broadcast                          (host replicates row)
    [4096:6144] inp2 * v_b                             (full rank, device)
    [6144:8192] (output_two * v_b) broadcast           (host replicates row)

Device computes everything TRANSPOSED (output features on partitions):
    i2T[o, l] = inp2[l, o] = sum_d W_in2^T[d,o] * input^T[d,l] + b[o]
so the bias is a per-partition scalar (ACT engine Identity+bias) and
prod = i2T * v[o] is a per-partition tensor_scalar on DVE. All matmul
operands are bf16 (inputs/weights pre-transposed + converted on host;
the PE program must be single-dtype: mixing bf16 with f32r transposes
faults the exec unit). Outputs stored bf16, upconverted on host:
rel err ~5e-3 << 2e-2.

idot rides as a [1,512] row accumulated over k with a [128,1] stationary
(w_in1 column), which makes e_bc construction transpose-free: idot row ->
K=1 broadcast matmul -> Exp (with accumulator for Z) straight into a
[128, 2048] replicated tile that serves both the q reduction (d on
partitions) and nothing else.

Sharding: pure data parallel, one batch element per NeuronCore (8 cores).

Scheduling: engine queues are strict in-order; emission order is the
schedule. mm tiles t=1..32 (g outer, oc inner); side events are spliced
between tiles no earlier than their producers. Reads are split between
the ACT ring (early: w2t/input groups 0-1) and the Pool SWDGE ring
(late: groups 2-3, memory, W_mem2) so no sequencer stalls compute;
writes own the SP ring.
"""

import numpy as np
import ml_dtypes

import concourse.bass as bass
import concourse.tile as tile
from concourse import bacc, mybir
from concourse.bass_utils import run_bass_kernel_spmd

F32 = mybir.dt.float32
BF16 = mybir.dt.bfloat16
AX = mybir.AxisListType
OP = mybir.AluOpType
EXP = mybir.ActivationFunctionType.Exp
IDENT = mybir.ActivationFunctionType.Identity
COPY = mybir.ActivationFunctionType.Copy

P = 128
BSZ, LD, LM, HID = 8, 2048, 512, 1024
KT = HID // P          # 8 hidden-dim chunks
LT = LD // P           # 16 l slices of 128
MT = LM // P           # 4 memory tiles
GT = 4                 # l groups of 512
GL = LD // GT          # 512
N_CORES = 8

_NC_CACHE = None


def _build_nc():
    nc = bacc.Bacc("TRN2", target_bir_lowering=False, num_devices=N_CORES)

    inT_d = nc.dram_tensor("inT", [HID, LD], BF16, kind="ExternalInput").ap()
    w2t_d = nc.dram_tensor("w2t", [HID, HID], BF16, kind="ExternalInput").ap()
    wm2t_d = nc.dram_tensor("wm2t", [HID, HID], BF16, kind="ExternalInput").ap()
    mem_d = nc.dram_tensor("memory", [LM, HID], BF16, kind="ExternalInput").ap()
    mask_d = nc.dram_tensor("mask", [1, LM], F32, kind="ExternalInput").ap()
    w1c_d = nc.dram_tensor("w1c", [P, KT], BF16, kind="ExternalInput").ap()
    wm1_d = nc.dram_tensor("wm1", [1, HID], BF16, kind="ExternalInput").ap()
    bi2c_d = nc.dram_tensor("bi2c", [P, KT], F32, kind="ExternalInput").ap()
    bm2c_d = nc.dram_tensor("bm2c", [P, KT], F32, kind="ExternalInput").ap()
    o1T_d = nc.dram_tensor("o1T", [HID, LD], BF16, kind="ExternalOutput").ap()
    o3T_d = nc.dram_tensor("o3T", [HID, LD], BF16, kind="ExternalOutput").ap()
    vrow_d = nc.dram_tensor("vrow", [P, KT], F32, kind="ExternalOutput").ap()
    urow_d = nc.dram_tensor("urow", [P, KT], F32, kind="ExternalOutput").ap()

    with tile.TileContext(nc) as tc:
        with (
            tc.tile_pool(name="const", bufs=1) as cpool,
            tc.tile_pool(name="w", bufs=1) as wpool,
            tc.tile_pool(name="inp", bufs=1) as inpool,
            tc.tile_pool(name="i2", bufs=1) as i2pool,
            tc.tile_pool(name="o3", bufs=1) as o3pool,
            tc.tile_pool(name="scr", bufs=2) as scrpool,
            tc.tile_pool(name="sm", bufs=1) as smpool,
            tc.tile_pool(name="psmm", bufs=4, space="PSUM") as psmm,
            tc.tile_pool(name="psbc", bufs=1, space="PSUM") as psbc,
            tc.tile_pool(name="psid", bufs=1, space="PSUM") as psid,
            tc.tile_pool(name="pssm", bufs=2, space="PSUM") as pssm,
        ):
            # ---------------- constants ----------------
            ones_col = cpool.tile([P, 1], BF16)    # partition-sum lhsT
            nc.vector.memset(ones_col[:], 1.0)
            ones_rowb = cpool.tile([1, P], BF16)   # partition-broadcast lhsT
            nc.vector.memset(ones_rowb[:], 1.0)

            # ---------------- reads ----------------
            # ACT ring, ordered by first use: bias col, W_in2^T half A
            # (k-halved for an earlier PE start), input group 0 (k-halved),
            # half B, group 1. Pool/SWDGE ring: the small tail tensors and
            # everything needed later (groups 2-3, memory, W_mem2^T).
            bi2c = wpool.tile([P, KT], F32, tag="bi2c")
            nc.scalar.dma_start(bi2c[:], bi2c_d[:])

            w2tp = {}  # (h, kh) -> [P, 4, GL]
            for kh in range(2):
                w = wpool.tile([P, 4, GL], BF16, tag=f"w2tp0{kh}",
                               name=f"w2tp0{kh}")
                nc.scalar.dma_start(
                    w[:],
                    w2t_d[kh * 512:(kh + 1) * 512, 0:GL].rearrange(
                        "(k p) x -> p k x", p=P),
                )
                w2tp[(0, kh)] = w
                t = inpool.tile([P, 4, GL], BF16, tag=f"inT0{kh}",
                                name=f"inT0{kh}")
                nc.scalar.dma_start(
                    t[:],
                    inT_d[kh * 512:(kh + 1) * 512, 0:GL].rearrange(
                        "(k p) x -> p k x", p=P),
                )
                w2tp[("in0", kh)] = t
            for kh in range(2):
                w = wpool.tile([P, 4, GL], BF16, tag=f"w2tp1{kh}",
                               name=f"w2tp1{kh}")
                nc.scalar.dma_start(
                    w[:],
                    w2t_d[kh * 512:(kh + 1) * 512, GL:2 * GL].rearrange(
                        "(k p) x -> p k x", p=P),
                )
                w2tp[(1, kh)] = w
            inTg1 = inpool.tile([P, KT, GL], BF16, tag="inTg1")
            nc.scalar.dma_start(
                inTg1[:],
                inT_d[:, GL:2 * GL].rearrange("(k p) x -> p k x", p=P),
            )

            # Pool/SWDGE ring
            w1c = wpool.tile([P, KT], BF16, tag="w1c")
            nc.gpsimd.dma_start(w1c[:], w1c_d[:])
            wm1_bc = wpool.tile([P, HID], BF16, tag="wm1bc")
            nc.gpsimd.dma_start(wm1_bc[:], wm1_d.to_broadcast([P, HID]))
            mask_col = wpool.tile([P, MT], F32, tag="maskc")
            nc.gpsimd.dma_start(mask_col[:],
                                mask_d.rearrange("1 (o p) -> p o", p=P))
            bm2c = wpool.tile([P, KT], F32, tag="bm2c")
            nc.gpsimd.dma_start(bm2c[:], bm2c_d[:])
            inTg23 = {}
            for g in (2, 3):
                t = inpool.tile([P, KT, GL], BF16, tag=f"inTg{g}",
                                name=f"inTg{g}")
                nc.gpsimd.dma_start(
                    t[:],
                    inT_d[:, g * GL:(g + 1) * GL].rearrange(
                        "(k p) x -> p k x", p=P),
                )
                inTg23[g] = t
            mem_t = wpool.tile([P, MT, HID], BF16, tag="memt")
            nc.gpsimd.dma_start(mem_t[:],
                                mem_d.rearrange("(j p) d -> p j d", p=P))
            wm2t_sb = wpool.tile([P, KT, HID], BF16, tag="wm2t")
            nc.gpsimd.dma_start(
                wm2t_sb[:], wm2t_d.rearrange("(k p) d -> p k d", p=P)
            )

            def w2ap(oc, k):
                h = oc // 4
                return w2tp[(h, k // 4)][:, k % 4,
                                         (oc % 4) * P:(oc % 4 + 1) * P]

            def inap(g, k):
                if g == 0:
                    return w2tp[("in0", k // 4)][:, k % 4, :]
                if g == 1:
                    return inTg1[:, k, :]
                return inTg23[g][:, k, :]

            # ---------------- persistent state ----------------
            i2T = [i2pool.tile([P, LD], BF16, tag=f"i2_{oc}", name=f"i2_{oc}")
                   for oc in range(KT)]
            o3sb = [o3pool.tile([P, LD], BF16, tag=f"o3_{oc}", name=f"o3_{oc}")
                    for oc in range(KT)]
            id_row = smpool.tile([1, LD], BF16, tag="idrow")
            e_bc = smpool.tile([P, LD], BF16, tag="ebc")
            zacc = smpool.tile([P, GT], F32, tag="zacc")
            z_col = smpool.tile([P, 1], F32, tag="zcol")
            rz_col = smpool.tile([P, 1], F32, tag="rzcol")
            q_part = smpool.tile([P, KT, GT], F32, tag="qpart")
            q_col = smpool.tile([P, KT], F32, tag="qcol")
            q_colb = smpool.tile([P, KT], BF16, tag="qcolb")
            s_mcol = smpool.tile([P, MT], F32, tag="smcol")
            msk = smpool.tile([P, MT], F32, tag="msk")
            e_s = smpool.tile([P, MT], BF16, tag="es")
            es_r = smpool.tile([P, 1], F32, tag="esr")
            es_rb = smpool.tile([P, 1], BF16, tag="esrb")
            zs_sb = smpool.tile([1, 1], BF16, tag="zssb")
            zsbc_sb = smpool.tile([P, 1], F32, tag="zsbc")
            rzs_col = smpool.tile([P, 1], F32, tag="rzs")
            p_col = smpool.tile([P, KT], BF16, tag="pcol")
            v_colf = smpool.tile([P, KT], F32, tag="vcolf")
            v_col = smpool.tile([P, KT], F32, tag="vcol")
            sq_col = smpool.tile([P, KT], F32, tag="sqcol")
            u_col = smpool.tile([P, KT], F32, tag="ucol")

            # ---------------- emitters ----------------
            def emit_mm(g, oc):
                ps = psmm.tile([P, GL], F32, tag="mm", name=f"mm{g}_{oc}")
                for k in range(KT):
                    nc.tensor.matmul(
                        ps[:], w2ap(oc, k), inap(g, k),
                        start=(k == 0), stop=(k == KT - 1),
                    )
                nc.scalar.activation(
                    i2T[oc][:, g * GL:(g + 1) * GL], ps[:], IDENT,
                    bias=bi2c[:, oc:oc + 1],
                )
                nc.sync.dma_start(
                    o1T_d[oc * P:(oc + 1) * P, g * GL:(g + 1) * GL],
                    i2T[oc][:, g * GL:(g + 1) * GL],
                )

            def emit_idot(g):
                # idot row for group g: [1, 512] accumulated over k with a
                # [128, 1] stationary — no transposes needed downstream.
                ps = psid.tile([1, GL], F32, tag="idr", name=f"idr{g}")
                for k in range(KT):
                    nc.tensor.matmul(
                        ps[:], w1c[:, k:k + 1], inap(g, k),
                        start=(k == 0), stop=(k == KT - 1),
                    )
                nc.scalar.copy(id_row[:, g * GL:(g + 1) * GL], ps[:])

            def emit_ebc(g):
                ps = psbc.tile([P, GL], F32, tag="bc", name=f"ebc{g}")
                nc.tensor.matmul(
                    ps[:], ones_rowb[:], id_row[:, g * GL:(g + 1) * GL],
                    start=True, stop=True,
                )
                nc.scalar.activation(
                    e_bc[:, g * GL:(g + 1) * GL], ps[:], EXP,
                    accum_out=zacc[:, g:g + 1],
                )

            def emit_q(g):
                # q_part[d, k, g] = sum_{l in g} inT[d,l] * e[l]
                for k in range(KT):
                    scr = scrpool.tile([P, GL], BF16, tag="scr", name=f"q{k}_{g}")
                    nc.vector.tensor_mul(
                        scr[:], inap(g, k), e_bc[:, g * GL:(g + 1) * GL]
                    )
                    nc.vector.tensor_reduce(
                        q_part[:, k, g:g + 1], scr[:], AX.X, OP.add
                    )

            def emit_smul(j):
                scrm = scrpool.tile([P, HID], BF16, tag="scrm", name=f"smul{j}")
                nc.vector.tensor_mul(scrm[:], mem_t[:, j, :], wm1_bc[:])
                nc.vector.tensor_reduce(
                    s_mcol[:, j:j + 1], scrm[:], AX.X, OP.add
                )

            def emit_es():
                nc.vector.tensor_scalar(msk[:], mask_col[:], -1.0, 1e30,
                                        OP.add, OP.mult)
                nc.vector.tensor_add(msk[:], msk[:], s_mcol[:])
                nc.scalar.activation(e_s[:], msk[:], EXP)

            def emit_zs():
                # Z_s = sum over all m of e_s; broadcast 1/Z_s to a column
                nc.vector.tensor_reduce(es_r[:], e_s[:], AX.X, OP.add)
                nc.scalar.copy(es_rb[:], es_r[:])
                ps = pssm.tile([P, 1], F32, tag="sm", name="zs")
                nc.tensor.matmul(ps[0:1, :], ones_col[:], es_rb[:],
                                 start=True, stop=True)
                nc.scalar.copy(zs_sb[:], ps[0:1, :])
                ps2 = pssm.tile([P, 1], F32, tag="sm", name="zsbc")
                nc.tensor.matmul(ps2[:], ones_rowb[:], zs_sb[:],
                                 start=True, stop=True)
                nc.scalar.copy(zsbc_sb[:], ps2[:])
                nc.vector.reciprocal(rzs_col[:], zsbc_sb[:])

            def emit_p(dc):
                ps = pssm.tile([P, 1], F32, tag="sm", name=f"p{dc}")
                for j in range(MT):
                    nc.tensor.matmul(
                        ps[:], mem_t[:, j, dc * P:(dc + 1) * P],
                        e_s[:, j:j + 1],
                        start=(j == 0), stop=(j == MT - 1),
                    )
                nc.scalar.activation(p_col[:, dc:dc + 1], ps[:], COPY,
                                     scale=rzs_col[:, 0:1])

            def emit_v(oc):
                ps = pssm.tile([P, 1], F32, tag="sm", name=f"v{oc}")
                for k in range(KT):
                    nc.tensor.matmul(
                        ps[:], wm2t_sb[:, k, oc * P:(oc + 1) * P],
                        p_col[:, k:k + 1],
                        start=(k == 0), stop=(k == KT - 1),
                    )
                nc.scalar.copy(v_colf[:, oc:oc + 1], ps[:])

            def emit_vfin():
                nc.vector.tensor_add(v_col[:], v_colf[:], bm2c[:])
                nc.sync.dma_start(vrow_d[:], v_col[:])

            def emit_qfin():
                for k in range(KT):
                    nc.vector.tensor_reduce(
                        q_col[:, k:k + 1], q_part[:, k, :], AX.X, OP.add
                    )
                nc.vector.tensor_copy(q_colb[:], q_col[:])
                nc.vector.tensor_reduce(z_col[:], zacc[:], AX.X, OP.add)
                nc.vector.reciprocal(rz_col[:], z_col[:])

            def emit_s(oc):
                ps = pssm.tile([P, 1], F32, tag="sm", name=f"s{oc}")
                for k in range(KT):
                    nc.tensor.matmul(
                        ps[:], w2ap(oc, k), q_colb[:, k:k + 1],
                        start=(k == 0), stop=(k == KT - 1),
                    )
                nc.scalar.copy(sq_col[:, oc:oc + 1], ps[:])

            def emit_u():
                nc.vector.tensor_scalar(u_col[:], bi2c[:], z_col[:, 0:1],
                                        None, OP.mult)
                nc.vector.tensor_add(u_col[:], u_col[:], sq_col[:])
                nc.vector.tensor_scalar(u_col[:], u_col[:], rz_col[:, 0:1],
                                        None, OP.mult)
                nc.vector.tensor_mul(u_col[:], u_col[:], v_col[:])
                nc.sync.dma_start(urow_d[:], u_col[:])

            def emit_prod(g, oc):
                nc.vector.tensor_scalar(
                    o3sb[oc][:, g * GL:(g + 1) * GL],
                    i2T[oc][:, g * GL:(g + 1) * GL],
                    v_col[:, oc:oc + 1], None, OP.mult,
                )
                if g == 1:
                    nc.sync.dma_start(
                        o3T_d[oc * P:(oc + 1) * P, 0:2 * GL],
                        o3sb[oc][:, 0:2 * GL],
                    )
                elif g == 3:
                    nc.sync.dma_start(
                        o3T_d[oc * P:(oc + 1) * P, 2 * GL:LD],
                        o3sb[oc][:, 2 * GL:LD],
                    )

            # ---------------- schedule ----------------
            # mm tiles t=1..32 (g outer, oc inner); side events spliced at
            # points where their producers (DMA arrivals or earlier events)
            # are guaranteed done so no engine's in-order stream stalls.
            def splice(t):
                if t in (2, 6, 10, 14):
                    emit_idot((t - 2) // 4)
                if t in (4, 8, 12, 16):
                    emit_ebc((t - 4) // 4)
                if t in (5, 9, 13, 17):
                    emit_q((t - 5) // 4)
                if t in (10, 11):
                    for j in range(2):
                        emit_smul((t - 10) * 2 + j)
                if t == 12:
                    emit_es()
                if t == 13:
                    emit_zs()
                if t in (14, 15):
                    for dc in range(4):
                        emit_p((t - 14) * 4 + dc)
                if t in (16, 17):
                    for oc in range(4):
                        emit_v((t - 16) * 4 + oc)
                if t == 18:
                    emit_qfin()
                    emit_vfin()
                    for oc in range(KT):
                        emit_prod(0, oc)
                if t == 19:
                    for oc in range(KT):
                        emit_prod(1, oc)
                if t in (20, 21):
                    for oc in range(4):
                        emit_s((t - 20) * 4 + oc)
                if t == 22:
                    emit_u()
                # prod(2, *) must wait until mm(2, 7) has been emitted (t=24)
                if t == 24:
                    for oc in range(KT):
                        emit_prod(2, oc)

            t = 0
            for g in range(GT):
                for oc in range(KT):
                    emit_mm(g, oc)
                    if g == GT - 1:
                        emit_prod(g, oc)
                    t += 1
                    splice(t)

    nc.finalize()
    return nc


def _get_nc():
    global _NC_CACHE
    if _NC_CACHE is None:
        _NC_CACHE = _build_nc()
    return _NC_CACHE


def kernel(**inputs) -> np.ndarray:
    nc = _get_nc()
    bf16 = ml_dtypes.bfloat16

    inp = np.asarray(inputs["input"], np.float32)
    mem = np.asarray(inputs["memory"], np.float32)
    mask = np.asarray(inputs["mask"], np.float32)
    w_in1 = np.asarray(inputs["w_in1"], np.float32).reshape(HID)
    w_mem1 = np.asarray(inputs["w_mem1"], np.float32).reshape(1, HID)
    W_in2 = np.asarray(inputs["W_in2"], np.float32)
    b_in2 = np.asarray(inputs["b_in2"], np.float32).reshape(HID)
    W_mem2 = np.asarray(inputs["W_mem2"], np.float32)
    b_mem2 = np.asarray(inputs["b_mem2"], np.float32).reshape(HID)

    w2t = W_in2.T.astype(bf16)
    wm2t = W_mem2.T.astype(bf16)
    w1c = w_in1.reshape(KT, P).T.astype(bf16)
    wm1 = w_mem1.astype(bf16)
    bi2c = np.ascontiguousarray(b_in2.reshape(KT, P).T)
    bm2c = np.ascontiguousarray(b_mem2.reshape(KT, P).T)

    in_maps = []
    for b in range(N_CORES):
        in_maps.append({
            "inT": inp[b].T.astype(bf16),
            "w2t": w2t,
            "wm2t": wm2t,
            "memory": mem[b].astype(bf16),
            "mask": np.ascontiguousarray(mask[b].reshape(1, LM)),
            "w1c": w1c,
            "wm1": wm1,
            "bi2c": bi2c,
            "bm2c": bm2c,
        })

    res = run_bass_kernel_spmd(nc, in_maps, core_ids=list(range(N_CORES)))

    out = np.empty((BSZ, 4 * LD, HID), np.float32)
    for b in range(N_CORES):
        r = res.results[b]
        out[b, 0:LD] = r["o1T"].T
        v = r["vrow"].T.reshape(HID).astype(np.float32)
        out[b, LD:2 * LD] = v
        out[b, 2 * LD:3 * LD] = r["o3T"].T
        u = r["urow"].T.reshape(HID).astype(np.float32)
        out[b, 3 * LD:4 * LD] = u
    return out
